# revision 2
# baseline (speedup 1.0000x reference)
"""Trainium2 Bass kernel for DepthFFN (histogram_binning).

Computes, for inputs
  image_features  (2, 32, 47, 156)  f32
  depth_logits    (2, 121, 47, 156) f32
  depth_maps      (2, 376, 1248)    f32
  depth_target_bin(2, 47, 156)      i32
the reference outputs
  frustum_features        (2, 32, 120, 47, 156) = softmax(logits)[:, :120] x image_features
  frustum_features_target (2, 32, 120, 47, 156) = onehot(bin)[:, :120]    x image_features
  pooled_depth            (2, 47, 156)          = sparse 8x8 avg pool of depth_maps

Sharding: 8 cores = (batch b in {0,1}) x (depth chunk dc in {0..3}, 30 bins each).
Each core writes its [32, 30, 47, 156] slice of both frustum tensors (d-major
row layout [30*32, 7332] so DMA stores are plain 2D), and redundantly computes
the (tiny) pooled output for both batches; the host takes core 0's copy.

On-chip plan per core (hw = H*W = 7332 on the free axis):
  - exp = Exp(logits) [121, 7332] (ACT), partition-summed via ones-matmul (PE),
    reciprocal of the sum via a DRAM-bounce reshape to [94, 78] (DVE exact
    reciprocal), broadcast back over 121 partitions via ones-matmul, and
    multiplied in place -> probs (DVE).
  - img_rep [128, 7332]: image rows replicated 4x (partition p holds channel
    p % 32) via a 0/1 selection matmul (PE) + PSUM->SBUF copy (ACT).
  - bin_rep [128, 7332]: target-bin row broadcast to 128 partitions (PE + ACT).
  - main loop over 8 d-tiles (4 depth bins x 32 channels = 128 partitions,
    last tile 2x32=64): probs rows replicated 32x each via selection matmul
    (PE -> PSUM), multiplied with img_rep (DVE) -> frustum_features tile;
    (bin_rep == d(p)) * img_rep fused in one scalar_tensor_tensor (DVE) ->
    frustum_features_target tile; both DMA'd out in ~1.3 MB chunks.
"""

import numpy as np

import concourse.bacc as bacc
import concourse.bass as bass
import concourse.mybir as mybir
import concourse.tile as tile
from concourse.bass_utils import run_bass_kernel_spmd

F32 = mybir.dt.float32
AF = mybir.ActivationFunctionType
OP = mybir.AluOpType
AX = mybir.AxisListType

B, C, D, DP1 = 2, 32, 120, 121
H, W = 47, 156
HW = H * W  # 7332
ND = 30  # depth bins per core
NCORES = 8
CW = 512  # psum / matmul moving-operand chunk width (one fp32 PSUM bank)
NCHUNK = (HW + CW - 1) // CW  # 15 (14 x 512 + 164)
# hw-column groups for output staging/DMA (chunk indices [0,5), [5,10), [10,15))
GROUPS = [(0, 5), (5, 10), (10, 15)]
# d-tiles per core: 7 x (4 bins) + 1 x (2 bins); partitions = bins*32
NT = 8


def _chunk(j):
    c0 = j * CW
    return c0, min(CW, HW - c0)


def build_program():
    nc = bacc.Bacc(
        "TRN2",
        target_bir_lowering=False,
        debug=False,
        num_devices=NCORES,
    )

    img_d = nc.dram_tensor("img", [C, HW], F32, kind="ExternalInput").ap()
    logits_d = nc.dram_tensor("logits", [DP1, HW], F32, kind="ExternalInput").ap()
    binf_d = nc.dram_tensor("binf", [1, HW], F32, kind="ExternalInput").ap()
    dvals_d = nc.dram_tensor("dvals", [128, NT], F32, kind="ExternalInput").ap()
    sel_d = nc.dram_tensor("sel", [DP1, ND * C], F32, kind="ExternalInput").ap()
    sel32_d = nc.dram_tensor("sel32", [C, 128], F32, kind="ExternalInput").ap()
    onescol_d = nc.dram_tensor("onescol", [DP1, 1], F32, kind="ExternalInput").ap()
    onesrow_d = nc.dram_tensor("onesrow", [1, 128], F32, kind="ExternalInput").ap()
    dmaps_d = nc.dram_tensor("dmaps", [94, 8, 1248], F32, kind="ExternalInput").ap()

    out_f_d = nc.dram_tensor("out_f", [ND * C, HW], F32, kind="ExternalOutput").ap()
    out_t_d = nc.dram_tensor("out_t", [ND * C, HW], F32, kind="ExternalOutput").ap()
    pooled_d = nc.dram_tensor("pooled", [94, 156], F32, kind="ExternalOutput").ap()

    with tile.TileContext(nc) as tc:
        with (
            tc.tile_pool(name="const", bufs=1) as constp,
            tc.tile_pool(name="big", bufs=1) as bigp,
            tc.tile_pool(name="psum", bufs=6, space="PSUM") as psp,
            tc.tile_pool(name="dram", bufs=1, space="DRAM") as dramp,
        ):
            # ---- constants ----
            sel_s = constp.tile([DP1, ND * C], F32, tag="sel")
            nc.sync.dma_start(sel_s, sel_d)
            sel32_s = constp.tile([C, 128], F32, tag="sel32")
            nc.sync.dma_start(sel32_s, sel32_d)
            onescol_s = constp.tile([DP1, 1], F32, tag="onescol")
            nc.sync.dma_start(onescol_s, onescol_d)
            onesrow_s = constp.tile([1, 128], F32, tag="onesrow")
            nc.sync.dma_start(onesrow_s, onesrow_d)
            dvals_s = constp.tile([128, NT], F32, tag="dvals")
            nc.sync.dma_start(dvals_s, dvals_d)

            # ---- long-lived big tiles ----
            exp_s = bigp.tile([DP1, HW], F32, tag="exp")  # logits -> exp -> probs
            imgrep_s = bigp.tile([128, HW], F32, tag="imgrep")
            binrep_s = bigp.tile([128, HW], F32, tag="binrep")

            nc.sync.dma_start(exp_s, logits_d)
            nc.scalar.activation(exp_s, exp_s, AF.Exp)

            sums_b = dramp.tile([1, 94, 78], F32, tag="sums")
            inv_b = dramp.tile([1, 94, 78], F32, tag="inv")
            sums_row = sums_b.rearrange("o p q -> o (p q)")  # [1, 7332] view
            inv_row = inv_b.rearrange("o p q -> o (p q)")

            with tc.tile_pool(name="ph0", bufs=1) as ph0:
                img_s = ph0.tile([C, HW], F32, tag="img")
                nc.sync.dma_start(img_s, img_d)
                binrow_s = ph0.tile([1, HW], F32, tag="binrow")
                nc.sync.dma_start(binrow_s, binf_d)

                # partition-sum of exp via ones-matmul; bounce to DRAM in
                # [94, 78] layout for a multi-lane exact reciprocal
                for j in range(NCHUNK):
                    c0, cw = _chunk(j)
                    ps = psp.tile([128, CW], F32, tag="ps")
                    nc.tensor.matmul(
                        ps[:1, :cw],
                        onescol_s,
                        exp_s[:, c0 : c0 + cw],
                        start=True,
                        stop=True,
                    )
                    s512 = ph0.tile([1, CW], F32, tag="s512", bufs=3)
                    nc.scalar.copy(s512[:, :cw], ps[:1, :cw])
                    nc.sync.dma_start(sums_row[:, c0 : c0 + cw], s512[:, :cw])

                r94 = ph0.tile([94, 78], F32, tag="r94")
                nc.sync.dma_start(r94, sums_b[0, :, :])
                nc.vector.reciprocal(r94, r94)
                nc.sync.dma_start(inv_b[0, :, :], r94)

                invrow_s = ph0.tile([1, HW], F32, tag="invrow")
                nc.sync.dma_start(invrow_s, inv_row)

                # probs = exp * (1/sum), broadcast over partitions via matmul
                for j in range(NCHUNK):
                    c0, cw = _chunk(j)
                    ps = psp.tile([128, CW], F32, tag="ps")
                    nc.tensor.matmul(
                        ps[:DP1, :cw],
                        onesrow_s[:1, :DP1],
                        invrow_s[:, c0 : c0 + cw],
                        start=True,
                        stop=True,
                    )
                    nc.vector.tensor_tensor(
                        exp_s[:, c0 : c0 + cw],
                        exp_s[:, c0 : c0 + cw],
                        ps[:DP1, :cw],
                        OP.mult,
                    )

                # img_rep: partition p <- img[p % 32]
                for j in range(NCHUNK):
                    c0, cw = _chunk(j)
                    ps = psp.tile([128, CW], F32, tag="ps")
                    nc.tensor.matmul(
                        ps[:, :cw],
                        sel32_s,
                        img_s[:, c0 : c0 + cw],
                        start=True,
                        stop=True,
                    )
                    nc.scalar.copy(imgrep_s[:, c0 : c0 + cw], ps[:, :cw])

                # bin_rep: bin row broadcast to 128 partitions
                for j in range(NCHUNK):
                    c0, cw = _chunk(j)
                    ps = psp.tile([128, CW], F32, tag="ps")
                    nc.tensor.matmul(
                        ps[:, :cw],
                        onesrow_s,
                        binrow_s[:, c0 : c0 + cw],
                        start=True,
                        stop=True,
                    )
                    nc.scalar.copy(binrep_s[:, c0 : c0 + cw], ps[:, :cw])

            # ---- main loop + pooling ----
            with (
                tc.tile_pool(name="outs", bufs=2) as outp,
                tc.tile_pool(name="poolx", bufs=1) as poolxp,
            ):
                for t in range(NT):
                    pt = 128 if t < NT - 1 else 64
                    m0 = 128 * t
                    for g0, g1 in GROUPS:
                        gc0 = g0 * CW
                        gc1 = min(g1 * CW, HW)
                        gw = gc1 - gc0
                        of = outp.tile([128, 5 * CW], F32, tag="of")
                        for j in range(g0, g1):
                            c0, cw = _chunk(j)
                            lo = c0 - gc0
                            ps = psp.tile([128, CW], F32, tag="ps")
                            nc.tensor.matmul(
                                ps[:pt, :cw],
                                sel_s[:, m0 : m0 + pt],
                                exp_s[:, c0 : c0 + cw],
                                start=True,
                                stop=True,
                            )
                            nc.vector.tensor_tensor(
                                of[:pt, lo : lo + cw],
                                ps[:pt, :cw],
                                imgrep_s[:pt, c0 : c0 + cw],
                                OP.mult,
                            )
                        nc.sync.dma_start(
                            out_f_d[m0 : m0 + pt, gc0:gc1], of[:pt, :gw]
                        )

                        ot = outp.tile([128, 5 * CW], F32, tag="ot")
                        nc.vector.scalar_tensor_tensor(
                            ot[:pt, :gw],
                            binrep_s[:pt, gc0:gc1],
                            dvals_s[:pt, t : t + 1],
                            imgrep_s[:pt, gc0:gc1],
                            OP.is_equal,
                            OP.mult,
                        )
                        nc.sync.dma_start(
                            out_t_d[m0 : m0 + pt, gc0:gc1], ot[:pt, :gw]
                        )

                # ---- sparse average pooling (both batches; 2 column halves) ----
                pooled_s = poolxp.tile([94, 156], F32, tag="pooled")
                for half in range(2):
                    h0 = half * 624
                    xp = poolxp.tile([94, 8, 624], F32, tag="xp")
                    nc.sync.dma_start(xp, dmaps_d[:, :, h0 : h0 + 624])
                    mk = poolxp.tile([94, 8, 624], F32, tag="mk")
                    # depth values are >= 0, so Sign(x) == (x != 0)
                    nc.scalar.activation(mk, xp, AF.Sign)
                    sv = poolxp.tile([94, 78], F32, tag="sv")
                    sm = poolxp.tile([94, 78], F32, tag="sm")
                    nc.vector.tensor_reduce(
                        sv,
                        xp.rearrange("p r (j q) -> p j r q", q=8),
                        axis=AX.XY,
                        op=OP.add,
                    )
                    nc.vector.tensor_reduce(
                        sm,
                        mk.rearrange("p r (j q) -> p j r q", q=8),
                        axis=AX.XY,
                        op=OP.add,
                    )
                    # ref: (sum/64) / (cnt/64 + 1e-10)
                    nc.vector.tensor_scalar(
                        sm, sm, 1.0 / 64.0, 1e-10, OP.mult, OP.add
                    )
                    nc.vector.reciprocal(sm, sm)
                    nc.vector.scalar_tensor_tensor(
                        pooled_s[:, h0 // 8 : h0 // 8 + 78],
                        sv,
                        1.0 / 64.0,
                        sm,
                        OP.mult,
                        OP.mult,
                    )
                nc.sync.dma_start(pooled_d, pooled_s)

    nc.finalize()
    return nc


_CACHE: dict = {}


def _get_program():
    if "nc" not in _CACHE:
        _CACHE["nc"] = build_program()
    return _CACHE["nc"]


def _make_in_maps(image_features, depth_logits, depth_maps, depth_target_bin):
    img = np.ascontiguousarray(np.asarray(image_features, np.float32)).reshape(
        B, C, HW
    )
    logits = np.ascontiguousarray(np.asarray(depth_logits, np.float32)).reshape(
        B, DP1, HW
    )
    binf = np.asarray(depth_target_bin).astype(np.float32).reshape(B, 1, HW)
    dmaps = np.ascontiguousarray(np.asarray(depth_maps, np.float32)).reshape(
        94, 8, 1248
    )

    sel32 = np.zeros((C, 128), np.float32)
    sel32[np.arange(128) % C, np.arange(128)] = 1.0
    onescol = np.ones((DP1, 1), np.float32)
    onesrow = np.ones((1, 128), np.float32)

    in_maps = []
    for core in range(NCORES):
        b, dc = divmod(core, 4)
        d0 = ND * dc
        # selection matrix: column m of d-tile t selects depth row d0+4t+m//32
        sel = np.zeros((DP1, ND * C), np.float32)
        dvals = np.full((128, NT), -7.0, np.float32)
        for t in range(NT):
            pt = 128 if t < NT - 1 else 64
            for m in range(pt):
                k = d0 + 4 * t + m // 32
                sel[k, 128 * t + m] = 1.0
            dvals[:pt, t] = d0 + 4 * t + np.arange(pt) // 32
        in_maps.append(
            {
                "img": img[b],
                "logits": logits[b],
                "binf": binf[b],
                "dvals": dvals,
                "sel": sel,
                "sel32": sel32,
                "onescol": onescol,
                "onesrow": onesrow,
                "dmaps": dmaps,
            }
        )
    return in_maps


def kernel(
    image_features,
    depth_logits,
    depth_maps,
    depth_target_bin,
    _trace=False,
    _tmpdir=None,
):
    nc = _get_program()
    in_maps = _make_in_maps(
        image_features, depth_logits, depth_maps, depth_target_bin
    )
    res = run_bass_kernel_spmd(
        nc,
        in_maps,
        core_ids=list(range(NCORES)),
        trace=_trace,
        tmpdir=_tmpdir,
    )
    _CACHE["last_results"] = res

    frustum = np.empty((B, C, D, H, W), np.float32)
    frustum_t = np.empty((B, C, D, H, W), np.float32)
    for core in range(NCORES):
        b, dc = divmod(core, 4)
        r = res.results[core]
        f = r["out_f"].reshape(ND, C, H, W).transpose(1, 0, 2, 3)
        ft = r["out_t"].reshape(ND, C, H, W).transpose(1, 0, 2, 3)
        frustum[b, :, ND * dc : ND * (dc + 1)] = f
        frustum_t[b, :, ND * dc : ND * (dc + 1)] = ft
    pooled = res.results[0]["pooled"].reshape(B, H, W).copy()
    return frustum, frustum_t, pooled
